# revision 2
# baseline (speedup 1.0000x reference)
"""DistMaps Trainium2 kernel v2 — tanh-space min with custom DVE rational tanh.

out[b, m, h, w] = tanh(2*sqrt(min_i d_i)), d_i = ((h-r_i)/5)^2 + ((w-c_i)/5)^2.

Per core (1 batch): out maps [2][128, 4*512] f32 init 1.0. Per point, a
[128, 2, 24] window strip around the point column is computed in d-space on
the PE (fp16 matmuls into PSUM), converted to t = tanh(2*sqrt(d)) via
ACT Sqrt + two custom DVE ops (rational approx, rel err ~5e-3), then
min-combined into the out map at a runtime offset (Pool engine). tanh is
monotonic so min commutes; t >= 1 in the far field so the 1.0-initialized
map acts as the clamp. No activation-table switches (only Sqrt/Square set),
stores start as soon as each half finishes.
"""
from contextlib import ExitStack

import numpy as np

import concourse.bass as bass
import concourse.tile as tile
from concourse import bacc, mybir

F32 = mybir.dt.float32
F16 = mybir.dt.float16
I32 = mybir.dt.int32
AF = mybir.ActivationFunctionType
OP = mybir.AluOpType

B = 8
H = W = 512
NPTS = 24
NPM = 12
NT = 4
WS = 24
CMARG = 11
BATCH = 4
NB = NPTS // BATCH
SW = 2 * WS          # strip free width per point (2 tiles x WS)
BW = BATCH * SW      # batch strip width
PEN = 3.0e4

# rational tanh constants (fit through exact fp32 pipeline, rel err 4.97e-3)
CA = 5.096068904130458
CB = 0.7337261350691104
CM0 = 324.08355704926106
CM1 = 25.882633249039866
CK = 0.011461100207676975
NRC = 2.0017324        # recip NR constant (via in1)
K0C = -0.23549792      # recip Chebyshev seed scale


def _register_ops():
    """Register the two custom DVE ops into concourse.dve_ops (idempotent)."""
    from concourse import dve_ops as dvo
    from concourse.dve_spec import AluOp, Bin, Src0, Src1, C0, C1, C2, Spec, lower, sq
    from concourse.dve_uop import DveOpSpec

    if "DIST_RECIP_DEN" in dvo._SUB_OPCODE_FOR_NAME:
        return (dvo.CUSTOM_DVE_SPECS and
                {o.name: o for o in dvo.OPS if o.name.startswith("DIST_")})

    _a1 = Src0 + C0
    _a2 = Src0 + C1
    _D = _a1 * _a2
    _nx = Bin(AluOp.BITWISE_NOT, _D, _D)
    _y0 = _nx * C2
    body1 = _y0 * (Src1 - _D * _y0)

    def ref1(in0, in1, c0, c1, c2):
        D = (in0.astype(np.float32) + np.float32(c0)) * (
            in0.astype(np.float32) + np.float32(c1))
        nx = (~D.view(np.int32)).view(np.float32)
        y0 = nx * np.float32(c2)
        return (y0 * (in1 - D * y0)).astype(np.float32)

    _v = sq(Src1)
    _w = sq(_v)
    _N = (_w + _v * C0) + C1
    body2 = ((_N * Src0) * Src1) * C2

    def ref2(in0, in1, c0, c1, c2):
        v = (in1.astype(np.float32)) ** 2
        N = v * v + v * np.float32(c0) + np.float32(c1)
        return (N * in0 * in1 * np.float32(c2)).astype(np.float32)

    made = {}
    for k, (name, spec_body, ref) in enumerate(
        (("DIST_RECIP_DEN", body1, ref1), ("DIST_TANH_NUM", body2, ref2))
    ):
        spec = Spec(body=spec_body, reference=ref)
        opcode = dvo._CUSTOM_DVE_ROW_BASE + len(dvo.OPS)
        uops = lower(spec, ver="v3")
        sha = DveOpSpec(name=name, opcode=opcode, uops=uops, rd1_en=True).sha("v3")
        op = dvo.DveOp(name, spec, subdim=False, uops_sha={"v3": sha})
        dvo.OPS.append(op)
        dvo._SUB_OPCODE_FOR_NAME[name] = opcode
        dvo.CUSTOM_DVE_SPECS[name] = spec
        made[name] = op
    return made


CBW = W + 2 * NT


def _make_consts():
    gw = np.arange(W, dtype=np.float16)
    cbh = np.zeros((NPTS, CBW), dtype=np.float16)
    cbh[:, 0:W] = gw[None, :]
    # tmc[i, 2t+c] = t - c  (match-selector pattern: is_equal vs t0 -> 1{t==t0+c})
    tc_pat = np.array([t - c for t in range(NT) for c in range(2)], dtype=np.float16)
    cbh[:, W:W + 2 * NT] = tc_pat[None, :]
    return {"cbh": cbh}


def _build():
    ops = _register_ops()
    OP1 = ops["DIST_RECIP_DEN"]
    OP2 = ops["DIST_TANH_NUM"]

    nc = bacc.Bacc("TRN2", target_bir_lowering=False, debug=False, num_devices=B)
    coords = nc.dram_tensor("coords", [NPTS, 3], F32, kind="ExternalInput").ap()
    cbhin = nc.dram_tensor("cbh", [NPTS, W + NPTS], F16, kind="ExternalInput").ap()
    y = nc.dram_tensor("y", [2, H, W], F32, kind="ExternalOutput").ap()

    with tile.TileContext(nc) as tc, ExitStack() as ctx:
        pool = ctx.enter_context(tc.tile_pool(name="sb", bufs=1))
        om_pool = ctx.enter_context(tc.tile_pool(name="om", bufs=1))
        s_pool = ctx.enter_context(tc.tile_pool(name="sstr", bufs=2))
        r_pool = ctx.enter_context(tc.tile_pool(name="rstr", bufs=2))
        t_pool = ctx.enter_context(tc.tile_pool(name="tstr", bufs=2))
        psum = ctx.enter_context(tc.tile_pool(name="ps", bufs=3, space="PSUM"))

        coords_sb = pool.tile([NPTS, 3], F32, tag="coords_sb")
        nc.sync.dma_start(coords_sb[:], coords[:])
        cbh = pool.tile([NPTS, W + NPTS], F16, tag="cbh_sb")
        nc.scalar.dma_start(cbh[:], cbhin[:])
        gw = cbh[:, 0:W]
        id24 = cbh[:, W:W + NPTS]

        ones1 = pool.tile([1, 128], F16, tag="ones1")
        nc.gpsimd.memset(ones1[:], 1.0)
        nrc2 = pool.tile([128, BW], F32, tag="nrc2")
        nc.gpsimd.memset(nrc2[:], NRC)

        outm = []
        for m in range(2):
            t = om_pool.tile([128, NT * W], F32, tag=f"out{m}")
            nc.gpsimd.memset(t[:], 1.0)
            outm.append(t)

        r = coords_sb[:, 0:1]
        c = coords_sb[:, 1:2]

        # penalty for invalid points (max(r,c) < 0)
        pen = pool.tile([NPTS, 1], F32, tag="pen")
        nc.vector.tensor_tensor(out=pen[:], in0=r, in1=c, op=OP.max)
        nc.vector.tensor_scalar(out=pen[:], in0=pen[:], scalar1=0.0,
                                scalar2=PEN, op0=OP.is_lt, op1=OP.mult)

        # t0 = (y>=1)+(y>=2), y = (r-12)/128 — exact float compares
        geo_pl = pool.tile([NPTS, 2], F32, tag="geo_pl")  # (t0, cs)
        geo_pe = pool.tile([NPTS, 3], F32, tag="geo_pe")  # (cs, 128*t0, 128*t0+128)
        yrow = pool.tile([NPTS, 1], F32, tag="yrow")
        nc.vector.tensor_scalar(out=yrow[:], in0=r, scalar1=-12.0,
                                scalar2=1.0 / 128.0, op0=OP.add, op1=OP.mult)
        nc.vector.tensor_scalar(out=geo_pl[:, 0:1], in0=yrow[:], scalar1=1.0,
                                scalar2=None, op0=OP.is_ge)
        nc.vector.scalar_tensor_tensor(out=geo_pl[:, 0:1], in0=yrow[:], scalar=2.0,
                                       in1=geo_pl[:, 0:1], op0=OP.is_ge, op1=OP.add)
        nc.vector.tensor_scalar(out=geo_pl[:, 1:2], in0=c, scalar1=float(-CMARG),
                                scalar2=0.0, op0=OP.add, op1=OP.max)
        nc.vector.tensor_scalar(out=geo_pl[:, 1:2], in0=geo_pl[:, 1:2],
                                scalar1=float(W - WS), scalar2=None, op0=OP.min)
        nc.vector.tensor_copy(geo_pe[:, 0:1], geo_pl[:, 1:2])
        nc.vector.tensor_scalar(out=geo_pe[:, 1:2], in0=geo_pl[:, 0:1],
                                scalar1=128.0, scalar2=None, op0=OP.mult)
        nc.vector.tensor_scalar(out=geo_pe[:, 2:3], in0=geo_pe[:, 1:2],
                                scalar1=128.0, scalar2=None, op0=OP.add)
        gpe = pool.tile([NPTS, 3], I32, tag="gpe")
        nc.vector.tensor_copy(gpe[:], geo_pe[:])
        gpl = pool.tile([NPTS, 2], I32, tag="gpl")
        nc.vector.tensor_copy(gpl[:], geo_pl[:])

        # row/col squared distances: rs[i, u] = ((u - r_i)/5)^2 + pen_i
        rdiff = pool.tile([NPTS, W], F16, tag="rdiff")
        nc.vector.tensor_scalar(out=rdiff[:], in0=gw, scalar1=r,
                                scalar2=None, op0=OP.subtract)
        cdiff = pool.tile([NPTS, W], F16, tag="cdiff")
        nc.vector.tensor_scalar(out=cdiff[:], in0=gw, scalar1=c,
                                scalar2=None, op0=OP.subtract)
        rs = pool.tile([NPTS, W], F16, tag="rs")
        nc.scalar.activation(rs[:], rdiff[:], AF.Square, scale=0.2)
        clut = pool.tile([NPTS, W], F16, tag="clut")
        nc.scalar.activation(clut[:], cdiff[:], AF.Square, scale=0.2)
        nc.vector.tensor_scalar(out=rs[:], in0=rs[:], scalar1=pen[:],
                                scalar2=None, op0=OP.add)

        for b in range(NB):
            m = b // (NB // 2)
            clb = psum.tile([128, BW], F32)
            clb4 = clb[:].rearrange("p (j c w) -> p j c w", j=BATCH, c=2)
            for j in range(BATCH):
                i = BATCH * b + j
                with nc.tensor.register() as rcs, nc.tensor.register() as r0, \
                     nc.tensor.register() as r1:
                    nc.tensor.reg_load([rcs, r0, r1], gpe[i:i + 1, 0:3])
                    csv = bass.make_scalar_value(rcs, min_val=0, max_val=W - WS)
                    w0 = bass.make_scalar_value(r0, min_val=0, max_val=384)
                    w1 = bass.make_scalar_value(r1, min_val=128, max_val=512)
                    rhsA = clut[i:i + 1, bass.ds(csv, WS)] \
                        .rearrange("p w -> p () w").to_broadcast((1, 2, WS))
                    nc.tensor.matmul(clb4[:, j], ones1[:], rhsA,
                                     start=True, stop=False)
                    rhsB = id24[:, i:i + 1].rearrange("p o -> p o ()") \
                        .to_broadcast((NPTS, 1, WS))
                    nc.tensor.matmul(clb4[:, j, 0:1], rs[:, bass.ds(w0, 128)],
                                     rhsB, start=False, stop=False,
                                     skip_group_check=True)
                    nc.tensor.matmul(clb4[:, j, 1:2], rs[:, bass.ds(w1, 128)],
                                     rhsB, start=False, stop=True,
                                     skip_group_check=True)

            sstrip = s_pool.tile([128, BW], F32, tag=f"ss{b%2}")
            nc.scalar.activation(sstrip[:], clb[:], AF.Sqrt, scale=4.0)
            rstrip = r_pool.tile([128, BW], F32, tag=f"rs{b%2}")
            nc.vector._custom_dve(OP1, out=rstrip[:], in0=clb[:], in1=nrc2[:],
                                  s0=CA, s1=CB, imm2=K0C)
            tstrip = t_pool.tile([128, BW], F32, tag=f"ts{b%2}")
            nc.vector._custom_dve(OP2, out=tstrip[:], in0=rstrip[:], in1=sstrip[:],
                                  s0=CM1, s1=CM0, imm2=CK)

            t4 = tstrip[:].rearrange("p (j c w) -> p j c w", j=BATCH, c=2)
            om4 = outm[m][:].rearrange("p (t w) -> p t w", t=NT)
            for j in range(BATCH):
                i = BATCH * b + j
                with nc.gpsimd.register() as rt, nc.gpsimd.register() as rc:
                    nc.gpsimd.reg_load([rt, rc], gpl[i:i + 1, 0:2])
                    t0v = bass.make_scalar_value(rt, min_val=0, max_val=2)
                    csv = bass.make_scalar_value(rc, min_val=0, max_val=W - WS)
                    dsl = om4[:, bass.ds(t0v, 2), bass.ds(csv, WS)]
                    nc.gpsimd.tensor_tensor(out=dsl, in0=t4[:, j], in1=dsl,
                                            op=OP.min)

            if b == NB // 2 - 1:
                nc.sync.dma_start(
                    y[0].rearrange("(t p) w -> p t w", t=NT), outm[0][:])
            elif b == NB - 1:
                nc.sync.dma_start(
                    y[1].rearrange("(t p) w -> p t w", t=NT), outm[1][:])

    nc.compile()
    return nc


_CACHE = {}


def _get_built():
    if "nc" not in _CACHE:
        _CACHE["nc"] = _build()
        _CACHE["consts"] = _make_consts()
    return _CACHE["nc"], _CACHE["consts"]


def kernel(x: np.ndarray, coords: np.ndarray) -> np.ndarray:
    assert x.shape == (B, 3, H, W), x.shape
    assert coords.shape == (B, NPTS, 3), coords.shape
    coords = np.ascontiguousarray(coords, dtype=np.float32)

    from concourse.bass_utils import run_bass_kernel_spmd
    nc, consts = _get_built()
    in_maps = [{"coords": coords[b], **consts} for b in range(B)]
    last_err = None
    for _attempt in range(3):
        try:
            res = run_bass_kernel_spmd(nc, in_maps, list(range(B)))
            break
        except Exception as e:
            last_err = e
    else:
        raise last_err
    out = np.stack([res.results[b]["y"] for b in range(B)])
    return out.astype(np.float32)
